# revision 1
# baseline (speedup 1.0000x reference)
"""Distributed 4-layer GAT for 8 Trainium2 NeuronCores (Bass/Tile).

Sharding: nodes partitioned across 8 cores (dst-sharding, 12500/core); edges
co-located with their dst, grouped into 128-dst windows; per-layer AllGather
of projected features h_ext (bf16, row = [h(512) | as | pad] = 640 elems);
aggregation = per-edge-tile matmuls with one-hot(dst)*w stationaries
accumulating in PSUM; softmax without max-subtraction (logits are small,
self-loops keep denominators positive); max pooling via transpose-mode
dma_gather, sum pooling via one-hot matmul; small AllGather of per-core
pooling partials + remap-gather combine; per-core MLP head on 125 graphs.
"""
import warnings

warnings.filterwarnings("ignore")

import numpy as np
import ml_dtypes

import concourse.bacc as bacc
import concourse.tile as tile
import concourse.bass as bass
import concourse.mybir as mybir
from concourse import bass_utils
from concourse.masks import make_identity

BF16 = mybir.dt.bfloat16
F32 = mybir.dt.float32
I16 = mybir.dt.int16
AF = mybir.ActivationFunctionType
ALU = mybir.AluOpType

NEG_SLOPE = 0.2


class Cfg:
    def __init__(self, N, E, G, F_IN, H, C, NC=8):
        self.N, self.E, self.G, self.F_IN, self.H, self.C, self.NC = N, E, G, F_IN, H, C, NC
        assert N % NC == 0 and G % NC == 0
        self.NPC = N // NC
        self.NPD = ((self.NPC + 127) // 128) * 128
        self.W = self.NPD // 128
        self.KC = H // 128
        self.KC1 = F_IN // 128
        self.ROW = H + 128                       # gather row elems (bf16)
        assert (self.ROW * 2) % 256 == 0
        self.TROWS = NC * self.NPD
        self.NQ = 4
        assert self.TROWS % self.NQ == 0
        self.QROWS = self.TROWS // self.NQ
        assert self.QROWS <= 32768
        self.TPQ = 2                             # tiles per (window, quarter)
        self.SPW = self.NQ * self.TPQ * 128      # slots per window
        self.TPW = self.NQ * self.TPQ            # tiles per window
        self.GPC = G // NC
        self.GW = 3
        self.groups = []
        w0 = 0
        while w0 < self.W:
            gw = min(self.GW, self.W - w0)
            self.groups.append((w0, gw))
            w0 += gw
        # filled by preprocess:
        self.S0 = None
        self.SG = None
        # debug knobs
        self.LAYERS = 4
        self.DO_POOL = True
        self.DO_AG = True
        self.DO_AGG = True
        self.AGG_LVL = 4


# ---------------------------------------------------------------------------
# host-side preprocessing
# ---------------------------------------------------------------------------

def _idx_block(a):
    """int index list -> [128, n/16] int16 (wrapped in 16 partitions, x8 replicas)."""
    a = np.asarray(a, np.int64)
    n = len(a)
    assert n % 16 == 0
    assert a.max(initial=0) < 32768 and a.min(initial=0) >= 0, (a.min(), a.max())
    blk = np.zeros((16, n // 16), np.int16)
    blk[np.arange(n) % 16, np.arange(n) // 16] = a.astype(np.int16)
    return np.ascontiguousarray(np.tile(blk, (8, 1)))


def preprocess(cfg, x, edge_index, batch_index, params):
    N, E, G, H = cfg.N, cfg.E, cfg.G, cfg.H
    NC, NPC, NPD, W = cfg.NC, cfg.NPC, cfg.NPD, cfg.W
    # self-loops are handled analytically on-device; only real edges here
    src = edge_index[0].astype(np.int64)
    dst = edge_index[1].astype(np.int64)
    srow = (src // NPC) * NPD + (src % NPC)      # padded global table row

    order = np.argsort(dst, kind="stable")
    dst_s, srow_s = dst[order], srow[order]

    bidx = np.asarray(batch_index, np.int64)
    counts = np.bincount(bidx, minlength=G)

    per_core = [dict() for _ in range(NC)]
    core_meta = []
    S0_need, SG_need = 0, 0
    for c in range(NC):
        lo_n, hi_n = c * NPC, (c + 1) * NPC
        sel = (dst_s >= lo_n) & (dst_s < hi_n)
        d_loc = dst_s[sel] - lo_n
        s_row = srow_s[sel]
        quarter = s_row // cfg.QROWS
        q_loc = s_row - quarter * cfg.QROWS
        win = d_loc // 128
        dl128 = d_loc % 128

        idx_q = [np.zeros(W * cfg.TPQ * 128, np.int64) for _ in range(cfg.NQ)]
        mt0 = np.zeros((W * cfg.TPW, 128, 128), np.float32)
        ad_idx = np.zeros(W * cfg.SPW, np.int64)
        for w in range(W):
            m_w = win == w
            for q in range(cfg.NQ):
                m = m_w & (quarter == q)
                ql, dl = q_loc[m], dl128[m]
                cnt = len(ql)
                assert cnt <= cfg.TPQ * 128, f"core{c} w{w} q{q}: {cnt} edges"
                base = w * cfg.TPQ * 128
                idx_q[q][base:base + cnt] = ql
                slot0 = w * cfg.SPW + q * cfg.TPQ * 128
                ad_idx[slot0:slot0 + cnt] = dl + w * 128
                t0 = w * cfg.TPW + q * cfg.TPQ
                if cnt:
                    ii = np.arange(cnt)
                    mt0[t0 + ii // 128, ii % 128, dl] = 1.0
        pc = per_core[c]
        pc["idxq"] = [_idx_block(ix) for ix in idx_q]
        pc["idx_ad"] = _idx_block(ad_idx)
        pc["mt0"] = np.ascontiguousarray(
            mt0.transpose(1, 0, 2).reshape(128, W * cfg.TPW * 128)
        ).astype(ml_dtypes.bfloat16)

        b_loc = bidx[lo_n:hi_n]
        g_lo, g_hi = int(b_loc[0]), int(b_loc[-1])
        SG_need = max(SG_need, g_hi - g_lo + 1)
        S0_need = max(S0_need, int(np.bincount(b_loc).max()))
        core_meta.append((g_lo, g_hi, b_loc))

    S0 = ((S0_need + 31) // 32) * 32     # x4 slots/piece stays %128 and <=1024
    SG = ((SG_need + 15) // 16) * 16
    assert SG <= 256, SG
    assert 4 * S0 <= 1024, S0
    cfg.S0, cfg.SG = S0, SG
    cfg.PIECE_SLOTS = 4
    cfg.POOL_PIECES = SG // 4

    for c in range(NC):
        g_lo, g_hi, b_loc = core_meta[c]
        ng = g_hi - g_lo + 1
        pc = per_core[c]
        max_idx = np.full(SG * S0, cfg.NPC, np.int64)    # zero row (pads)
        sum_idx = np.full(SG * S0, cfg.NPC, np.int64)
        starts = np.searchsorted(b_loc, np.arange(g_lo, g_hi + 2))
        for s in range(ng):
            a, b = int(starts[s]), int(starts[s + 1])
            rows = np.arange(a, b)
            max_idx[s * S0:s * S0 + (b - a)] = rows
            if b > a:
                max_idx[s * S0 + (b - a):(s + 1) * S0] = a   # repeat first node
            sum_idx[s * S0:s * S0 + (b - a)] = rows
        pc["idx_pmax"] = _idx_block(max_idx)
        pc["idx_psum"] = _idx_block(sum_idx)

        Smat = np.zeros((W, 128, SG), np.float32)
        ii = np.arange(cfg.NPC)
        Smat[ii // 128, ii % 128, b_loc - g_lo] = 1.0
        pc["smat"] = np.ascontiguousarray(
            Smat.transpose(1, 0, 2).reshape(128, W * SG)).astype(ml_dtypes.bfloat16)

        own0 = c * cfg.GPC
        comb = np.zeros(8 * 128, np.int64)
        DUMMY_NEG, DUMMY_ZERO = NC * SG, NC * SG + 1
        for r in range(NC):
            glr, ghr, _ = core_meta[r]
            for j in range(cfg.GPC):
                g = own0 + j
                if glr <= g <= ghr:
                    comb[r * 128 + j] = r * SG + (g - glr)
                elif r == 0:
                    comb[r * 128 + j] = DUMMY_ZERO
                else:
                    comb[r * 128 + j] = DUMMY_NEG
        # empty graphs: max must come out 0 (reference: where(isfinite, 0))
        for j in range(cfg.GPC):
            if counts[own0 + j] == 0:
                for r in range(NC):
                    comb[r * 128 + j] = DUMMY_ZERO
        pc["idx_comb"] = _idx_block(comb)
        inv_cnt = np.zeros((128, 1), np.float32)
        inv_cnt[:cfg.GPC, 0] = 1.0 / np.maximum(counts[own0:own0 + cfg.GPC], 1)
        pc["inv_cnt"] = inv_cnt

        x1 = np.zeros((NPD, cfg.F_IN), np.float32)
        x1[:NPC] = x[c * NPC:(c + 1) * NPC]
        pc["x1T"] = np.ascontiguousarray(x1.T.astype(ml_dtypes.bfloat16))

    shared = {}
    for l in range(1, 5):
        Wl = np.asarray(params[f"W{l}"], np.float64)
        a_s = np.asarray(params[f"a{l}s"], np.float64)
        a_d = np.asarray(params[f"a{l}d"], np.float64)
        wext = np.concatenate([Wl, (Wl @ a_s)[:, None], (Wl @ a_d)[:, None]], 1)
        shared[f"wext{l}"] = np.ascontiguousarray(
            wext.astype(np.float32).astype(ml_dtypes.bfloat16))
        shared[f"brep{l}"] = np.ascontiguousarray(
            np.tile(np.asarray(params[f"b{l}"], np.float32)[None, :], (128, 1)))
    shared["lin1W"] = np.ascontiguousarray(
        np.asarray(params["lin1_W"], np.float32).astype(ml_dtypes.bfloat16))
    shared["lin1b"] = np.ascontiguousarray(
        np.tile(np.asarray(params["lin1_b"], np.float32)[None, :], (128, 1)))
    l2p = np.zeros((H, 16), np.float32)
    l2p[:, :cfg.C] = np.asarray(params["lin2_W"], np.float32)
    shared["lin2W"] = np.ascontiguousarray(l2p.astype(ml_dtypes.bfloat16))
    b2 = np.zeros((128, 16), np.float32)
    b2[:, :cfg.C] = np.tile(np.asarray(params["lin2_b"], np.float32)[None, :], (128, 1))
    shared["lin2b"] = b2

    in_maps = []
    for c in range(NC):
        pc = per_core[c]
        m = dict(x1T=pc["x1T"], idx_ad=pc["idx_ad"], idx_pmax=pc["idx_pmax"],
                 idx_psum=pc["idx_psum"], idx_comb=pc["idx_comb"],
                 inv_cnt=pc["inv_cnt"], smat=pc["smat"], mt0=pc["mt0"])
        for q in range(cfg.NQ):
            m[f"idxq{q}"] = pc["idxq"][q]
        m.update(shared)
        in_maps.append(m)
    return in_maps


# ---------------------------------------------------------------------------
# device kernel
# ---------------------------------------------------------------------------

def build_kernel(cfg):
    NC, W, H, ROW, NPD = cfg.NC, cfg.W, cfg.H, cfg.ROW, cfg.NPD
    H2 = 2 * H
    nc = bacc.Bacc("TRN2", target_bir_lowering=False, debug=False,
                   enable_asserts=False, num_devices=NC)

    t_x1T = nc.dram_tensor("x1T", [cfg.F_IN, NPD], BF16, kind="ExternalInput")
    t_wext, t_brep = [], []
    for l in range(1, 5):
        F_l = cfg.F_IN if l == 1 else H
        t_wext.append(nc.dram_tensor(f"wext{l}", [F_l, H + 2], BF16, kind="ExternalInput"))
        t_brep.append(nc.dram_tensor(f"brep{l}", [128, H], F32, kind="ExternalInput"))
    t_idxq = [nc.dram_tensor(f"idxq{q}", [128, W * cfg.TPQ * 8], I16, kind="ExternalInput")
              for q in range(cfg.NQ)]
    t_idx_ad = nc.dram_tensor("idx_ad", [128, W * cfg.SPW // 16], I16, kind="ExternalInput")
    t_mt0 = nc.dram_tensor("mt0", [128, W * cfg.TPW * 128], BF16, kind="ExternalInput")
    t_smat = nc.dram_tensor("smat", [128, W * cfg.SG], BF16, kind="ExternalInput")
    t_idx_pmax = nc.dram_tensor("idx_pmax", [128, cfg.SG * cfg.S0 // 16], I16, kind="ExternalInput")
    t_idx_psum = nc.dram_tensor("idx_psum", [128, cfg.SG * cfg.S0 // 16], I16, kind="ExternalInput")
    t_idx_comb = nc.dram_tensor("idx_comb", [128, 8 * 128 // 16], I16, kind="ExternalInput")
    t_inv_cnt = nc.dram_tensor("inv_cnt", [128, 1], F32, kind="ExternalInput")
    t_lin1W = nc.dram_tensor("lin1W", [H2, H], BF16, kind="ExternalInput")
    t_lin1b = nc.dram_tensor("lin1b", [128, H], F32, kind="ExternalInput")
    t_lin2W = nc.dram_tensor("lin2W", [H, 16], BF16, kind="ExternalInput")
    t_lin2b = nc.dram_tensor("lin2b", [128, 16], F32, kind="ExternalInput")
    t_out = nc.dram_tensor("out", [128, 16], F32, kind="ExternalOutput")

    t_hown = nc.dram_tensor("hown", [NPD, ROW], BF16, kind="Internal")
    t_hfull = nc.dram_tensor("hfull", [NC * NPD, ROW], BF16, kind="Internal",
                             addr_space="Shared")
    t_xnext = nc.dram_tensor("xnext", [NPD + 128, H], BF16, kind="Internal")
    t_adrep = nc.dram_tensor("adrep", [NPD, 128], BF16, kind="Internal")
    t_pool_x = nc.dram_tensor("pool_x", [NPD + 128, H], BF16, kind="Internal")
    t_part = nc.dram_tensor("part", [cfg.SG, H2], F32, kind="Internal")
    t_part_ag = nc.dram_tensor("part_ag", [NC * cfg.SG + 128, H2], F32,
                               kind="Internal", addr_space="Shared")

    NBLK = 7 if (W % 7 == 0) else 1
    TPB = W // NBLK
    NSG = (cfg.SG + 127) // 128
    CH = H // 128
    LAST_ROWS = cfg.NPC - 128 * (W - 1)   # valid rows in last window

    with tile.TileContext(nc) as tc:
        import contextlib
        with contextlib.ExitStack() as ctx:
            const = ctx.enter_context(tc.tile_pool(name="const", bufs=1))
            sb_w = ctx.enter_context(tc.tile_pool(name="weights", bufs=1))
            sb_xt = ctx.enter_context(tc.tile_pool(name="xt", bufs=2))
            sb_dense = ctx.enter_context(tc.tile_pool(name="dense", bufs=3))
            sb_g = ctx.enter_context(tc.tile_pool(name="gather", bufs=2))
            sb_mt = ctx.enter_context(tc.tile_pool(name="mt", bufs=2))
            sb_win = ctx.enter_context(tc.tile_pool(name="win", bufs=2))
            ps_big = ctx.enter_context(tc.tile_pool(name="psb", bufs=3, space="PSUM"))
            ps_sm = ctx.enter_context(tc.tile_pool(name="pssm", bufs=3, space="PSUM"))
            ps_t = ctx.enter_context(tc.tile_pool(name="pst", bufs=2, space="PSUM"))

            ones_col = const.tile([128, 1], BF16)
            nc.gpsimd.memset(ones_col[:], 1.0)
            zeros_sb = const.tile([128, H], BF16)
            nc.gpsimd.memset(zeros_sb[:], 0.0)
            ident = const.tile([128, 128], F32)
            make_identity(nc, ident[:])
            ident_bf = const.tile([128, 128], BF16)
            nc.vector.tensor_copy(out=ident_bf[:], in_=ident[:])
            as_sb = const.tile([128, W], F32)
            ad_sb = const.tile([128, W], F32)


            for l in range(1, cfg.LAYERS + 1):
                F_l = cfg.F_IN if l == 1 else H
                KCl = F_l // 128
                wsb = sb_w.tile([128, KCl * (H + 2)], BF16, tag="wsb")
                for k in range(KCl):
                    nc.sync.dma_start(out=wsb[:, k * (H + 2):(k + 1) * (H + 2)],
                                      in_=t_wext[l - 1][k * 128:(k + 1) * 128, :])
                brep = sb_w.tile([128, H], F32, tag="brep")
                nc.sync.dma_start(out=brep[:], in_=t_brep[l - 1][:, :])

                # ---------------- dense: h_ext = x @ [W | was | wad]
                for b in range(NBLK):
                    xt = sb_xt.tile([128, KCl * TPB * 128], BF16, tag="xt")
                    if l > 1:
                        for k in range(KCl):
                            nc.sync.dma_start(
                                out=xt[:, k * TPB * 128:(k + 1) * TPB * 128],
                                in_=t_xnext[b * TPB * 128:(b + 1) * TPB * 128,
                                            k * 128:(k + 1) * 128],
                                transpose=True)
                    else:
                        nc.sync.dma_start(
                            out=xt[:, 0:TPB * 128],
                            in_=t_x1T[:, b * TPB * 128:(b + 1) * TPB * 128])
                    for i in range(TPB):
                        ti = b * TPB + i

                        def lhsT_of(k):
                            return xt[:, (k * TPB + i) * 128:(k * TPB + i + 1) * 128]

                        ph = ps_big.tile([128, H], F32, tag="pbig")
                        for k in range(KCl):
                            nc.tensor.matmul(ph[:], lhsT_of(k),
                                             wsb[:, k * (H + 2):k * (H + 2) + H],
                                             start=(k == 0), stop=(k == KCl - 1))
                        psc = ps_sm.tile([128, 16], F32, tag="psm")
                        for k in range(KCl):
                            nc.tensor.matmul(psc[:, 0:2], lhsT_of(k),
                                             wsb[:, k * (H + 2) + H:(k + 1) * (H + 2)],
                                             start=(k == 0), stop=(k == KCl - 1))
                        hext = sb_dense.tile([128, H + 1], BF16, tag="hext")
                        nc.vector.tensor_copy(out=hext[:, 0:H], in_=ph[:])
                        nc.vector.tensor_copy(out=hext[:, H:H + 1], in_=psc[:, 0:1])
                        nc.vector.tensor_copy(out=as_sb[:, ti:ti + 1], in_=psc[:, 0:1])
                        nc.vector.tensor_copy(out=ad_sb[:, ti:ti + 1], in_=psc[:, 1:2])
                        nc.sync.dma_start(out=t_hown[ti * 128:(ti + 1) * 128, 0:H + 1],
                                          in_=hext[:])
                        adr = sb_dense.tile([128, 128], BF16, tag="adr")
                        nc.vector.tensor_copy(out=adr[:],
                                              in_=psc[:, 1:2].to_broadcast([128, 128]))
                        nc.sync.dma_start(out=t_adrep[ti * 128:(ti + 1) * 128, :],
                                          in_=adr[:])

                # ---------------- AllGather
                if cfg.DO_AG:
                    nc.gpsimd.collective_compute(
                        "AllGather", ALU.bypass,
                        replica_groups=[list(range(NC))],
                        ins=[t_hown[:, :]], outs=[t_hfull[:, :]])
                else:
                    for _c in range(NC):
                        nc.sync.dma_start(
                            out=t_hfull[_c * NPD:(_c + 1) * NPD, :],
                            in_=t_hown[:, :])

                # ---------------- aggregation
                dst_t = t_pool_x if l == cfg.LAYERS else t_xnext
                agg_groups = cfg.groups if cfg.DO_AGG else []
                for (w0, gw) in agg_groups:
                    gbufs = []
                    for q in range(cfg.NQ):
                        iq = sb_mt.tile([128, cfg.GW * cfg.TPQ * 8], I16, tag="iq")
                        nc.sync.dma_start(
                            out=iq[:, 0:gw * cfg.TPQ * 8],
                            in_=t_idxq[q][:, w0 * cfg.TPQ * 8:(w0 + gw) * cfg.TPQ * 8])
                        gq = sb_g.tile([128, cfg.GW * cfg.TPQ, ROW], BF16, tag=f"g{q}")
                        nc.gpsimd.dma_gather(
                            out_ap=gq[:, 0:gw * cfg.TPQ, :],
                            in_ap=t_hfull[q * cfg.QROWS:(q + 1) * cfg.QROWS, :],
                            idxs_ap=iq[:, 0:gw * cfg.TPQ * 8],
                            num_idxs=gw * cfg.TPQ * 128,
                            num_idxs_reg=gw * cfg.TPQ * 128,
                            elem_size=ROW)
                        gbufs.append(gq)
                    adg = sb_g.tile([128, cfg.GW * cfg.TPW, 128], BF16, tag="adg")
                    ia = sb_mt.tile([128, cfg.GW * cfg.SPW // 16], I16, tag="ia")
                    nc.sync.dma_start(
                        out=ia[:, 0:gw * cfg.SPW // 16],
                        in_=t_idx_ad[:, w0 * cfg.SPW // 16:(w0 + gw) * cfg.SPW // 16])
                    for lw in range(gw):
                        # dma_gather is limited to <=1024 indices per call
                        nc.gpsimd.dma_gather(
                            out_ap=adg[:, lw * cfg.TPW:(lw + 1) * cfg.TPW, :],
                            in_ap=t_adrep[:, :],
                            idxs_ap=ia[:, lw * cfg.SPW // 16:(lw + 1) * cfg.SPW // 16],
                            num_idxs=cfg.SPW, num_idxs_reg=cfg.SPW,
                            elem_size=128)
                    mt0g = sb_mt.tile([128, cfg.GW * cfg.TPW * 128], BF16, tag="mt0g")
                    nc.sync.dma_start(
                        out=mt0g[:, 0:gw * cfg.TPW * 128],
                        in_=t_mt0[:, w0 * cfg.TPW * 128:(w0 + gw) * cfg.TPW * 128])

                    TPW1 = cfg.TPW + 1
                    if cfg.AGG_LVL < 2:
                        continue
                    wbuf = sb_win.tile([128, cfg.GW * TPW1, 1], F32, tag="wbuf")
                    for lw in range(gw):
                        for q in range(cfg.NQ):
                            o0 = lw * TPW1 + q * cfg.TPQ
                            nc.vector.tensor_tensor(
                                out=wbuf[:, o0:o0 + cfg.TPQ, :],
                                in0=gbufs[q][:, lw * cfg.TPQ:(lw + 1) * cfg.TPQ, H:H + 1],
                                in1=adg[:, lw * cfg.TPW + q * cfg.TPQ:
                                        lw * cfg.TPW + (q + 1) * cfg.TPQ, 0:1],
                                op=ALU.add)
                        nc.vector.tensor_tensor(
                            out=wbuf[:, lw * TPW1 + cfg.TPW, 0:1],
                            in0=as_sb[:, w0 + lw:w0 + lw + 1],
                            in1=ad_sb[:, w0 + lw:w0 + lw + 1], op=ALU.add)
                    wb2 = sb_win.tile([128, cfg.GW * TPW1, 1], F32, tag="wb2")
                    nc.vector.tensor_scalar(
                        out=wb2[:, 0:gw * TPW1, :], in0=wbuf[:, 0:gw * TPW1, :],
                        scalar1=NEG_SLOPE, scalar2=None, op0=ALU.mult)
                    nc.vector.tensor_tensor(
                        out=wbuf[:, 0:gw * TPW1, :], in0=wbuf[:, 0:gw * TPW1, :],
                        in1=wb2[:, 0:gw * TPW1, :], op=ALU.max)
                    nc.scalar.activation(wbuf[:, 0:gw * TPW1, :],
                                         wbuf[:, 0:gw * TPW1, :], AF.Exp)

                    if cfg.AGG_LVL < 3:
                        continue
                    for lw in range(gw):
                        w = w0 + lw
                        po = ps_big.tile([128, H], F32, tag="pbig")
                        pd = ps_sm.tile([128, 16], F32, tag="psm")
                        # self-loop first: diag(w_self) @ h_own[window]
                        hw = sb_dense.tile([128, H], BF16, tag="hw")
                        nc.sync.dma_start(out=hw[:],
                                          in_=t_hown[w * 128:(w + 1) * 128, 0:H])
                        mts = sb_mt.tile([128, 128], BF16, tag="mt")
                        nc.vector.tensor_scalar(
                            out=mts[:], in0=ident_bf[:],
                            scalar1=wbuf[:, lw * TPW1 + cfg.TPW, 0:1],
                            scalar2=None, op0=ALU.mult)
                        nc.tensor.matmul(po[:], mts[:], hw[:], start=True, stop=False)
                        nc.tensor.matmul(pd[:, 0:1], mts[:], ones_col[:],
                                         start=True, stop=False)
                        for t in range(cfg.TPW):
                            q, sub = t // cfg.TPQ, t % cfg.TPQ
                            mt = sb_mt.tile([128, 128], BF16, tag="mt")
                            nc.vector.tensor_scalar(
                                out=mt[:],
                                in0=mt0g[:, (lw * cfg.TPW + t) * 128:
                                         (lw * cfg.TPW + t + 1) * 128],
                                scalar1=wbuf[:, lw * TPW1 + t, 0:1],
                                scalar2=None, op0=ALU.mult)
                            nc.tensor.matmul(po[:], mt[:],
                                             gbufs[q][:, lw * cfg.TPQ + sub, 0:H],
                                             start=False, stop=(t == cfg.TPW - 1))
                            nc.tensor.matmul(pd[:, 0:1], mt[:], ones_col[:],
                                             start=False, stop=(t == cfg.TPW - 1))
                        if cfg.AGG_LVL < 4:
                            continue
                        rec = sb_win.tile([128, 1], F32, tag="rec")
                        nc.vector.reciprocal(rec[:], pd[:, 0:1])
                        xn = sb_win.tile([128, H], BF16, tag="xn")
                        nc.vector.scalar_tensor_tensor(
                            out=xn[:], in0=po[:], scalar=rec[:, 0:1], in1=brep[:],
                            op0=ALU.mult, op1=ALU.add)
                        nc.scalar.activation(xn[:], xn[:], AF.Tanh)
                        nr = LAST_ROWS if w == W - 1 else 128
                        nc.sync.dma_start(out=dst_t[w * 128:w * 128 + nr, 0:H],
                                          in_=xn[0:nr, :])
                if not cfg.DO_AGG:
                    for w in range(W):
                        nc.sync.dma_start(out=dst_t[w * 128:(w + 1) * 128, 0:H],
                                          in_=t_hown[w * 128:(w + 1) * 128, 0:H])
                nc.sync.dma_start(out=dst_t[cfg.NPC:cfg.NPC + 128, 0:H],
                                  in_=zeros_sb[:])

            if not cfg.DO_POOL:
                dbg = sb_win.tile([128, 16], F32, tag="outt")
                nc.gpsimd.memset(dbg[:], 0.0)
                nc.sync.dma_start(out=t_out[:, :], in_=dbg[:])
            else:
                # ---------------- pooling ----------------
                psums = []
                for _sgc in range(NSG):
                    pss_t = ps_big.tile([128, H], F32, tag="pbig", name=f"pss{_sgc}")
                    psums.append(pss_t)
                for i in range(W):
                    smt = sb_dense.tile([128, cfg.SG], BF16, tag="smt")
                    nc.sync.dma_start(out=smt[:], in_=t_smat[:, i * cfg.SG:(i + 1) * cfg.SG])
                    xtile = sb_dense.tile([128, H], BF16, tag="hext")
                    nc.sync.dma_start(out=xtile[:],
                                      in_=t_pool_x[i * 128:(i + 1) * 128, 0:H])
                    for sgc in range(NSG):
                        cols = min(128, cfg.SG - sgc * 128)
                        nc.tensor.matmul(
                            psums[sgc][0:cols, :],
                            smt[:, sgc * 128:sgc * 128 + cols],
                            xtile[:], start=(i == 0), stop=(i == W - 1))
                for sgc in range(NSG):
                    cols = min(128, cfg.SG - sgc * 128)
                    ssb = sb_win.tile([128, H], F32, tag="ssb")
                    nc.vector.tensor_copy(out=ssb[0:cols, :], in_=psums[sgc][0:cols, :])
                    nc.sync.dma_start(out=t_part[sgc * 128:sgc * 128 + cols, H:H2],
                                      in_=ssb[0:cols, :])

                idx_pm_sb = const.tile([128, cfg.SG * cfg.S0 // 16], I16)
                nc.sync.dma_start(out=idx_pm_sb[:], in_=t_idx_pmax[:, :])
                maxT = const.tile([128, CH * cfg.SG], F32)
                PS = cfg.PIECE_SLOTS
                for piece in range(cfg.POOL_PIECES):
                    n_idx = PS * cfg.S0
                    gt = sb_g.tile([128, CH, n_idx], BF16, tag="g0")
                    nc.gpsimd.dma_gather(
                        out_ap=gt[:],
                        in_ap=t_pool_x[:, :],
                        idxs_ap=idx_pm_sb[:, piece * n_idx // 16:(piece + 1) * n_idx // 16],
                        num_idxs=n_idx, num_idxs_reg=n_idx,
                        elem_size=H, transpose=True)
                    for ch in range(CH):
                        nc.vector.tensor_reduce(
                            out=maxT[:, ch * cfg.SG + piece * PS:
                                     ch * cfg.SG + (piece + 1) * PS],
                            in_=gt[:, ch, :].rearrange("p (s j) -> p s j", j=cfg.S0),
                            axis=mybir.AxisListType.X, op=ALU.max)
                for sgc in range(NSG):
                    cols = min(128, cfg.SG - sgc * 128)
                    for ch in range(CH):
                        pt = ps_t.tile([128, 128], F32, tag="pt")
                        nc.tensor.matmul(
                            pt[0:cols, 0:128],
                            maxT[:, ch * cfg.SG + sgc * 128:ch * cfg.SG + sgc * 128 + cols],
                            ident[:], is_transpose=True, start=True, stop=True)
                        mtile = sb_win.tile([128, 128], F32, tag="mtile")
                        nc.vector.tensor_copy(out=mtile[0:cols, :], in_=pt[0:cols, 0:128])
                        nc.sync.dma_start(
                            out=t_part[sgc * 128:sgc * 128 + cols, ch * 128:(ch + 1) * 128],
                            in_=mtile[0:cols, :])

                nc.gpsimd.collective_compute(
                    "AllGather", ALU.bypass,
                    replica_groups=[list(range(NC))],
                    ins=[t_part[:, :]], outs=[t_part_ag[0:NC * cfg.SG, :]])
                zr = sb_win.tile([128, H2], F32, tag="zr")
                nc.gpsimd.memset(zr[:, 0:H], -1e30)
                nc.gpsimd.memset(zr[:, H:H2], 0.0)
                nc.sync.dma_start(out=t_part_ag[NC * cfg.SG:NC * cfg.SG + 1, :],
                                  in_=zr[0:1, :])
                zrz = sb_win.tile([128, H2], F32, tag="zrz")
                nc.gpsimd.memset(zrz[:], 0.0)
                nc.sync.dma_start(out=t_part_ag[NC * cfg.SG + 1:NC * cfg.SG + 2, :],
                                  in_=zrz[0:1, :])

                idx_cb_sb = const.tile([128, 8 * 128 // 16], I16)
                nc.sync.dma_start(out=idx_cb_sb[:], in_=t_idx_comb[:, :])
                z = sb_win.tile([128, H2], F32, tag="z")
                for half in range(2):
                    cmb = sb_g.tile([128, 8, H], F32, tag="g1")
                    nc.gpsimd.dma_gather(
                        out_ap=cmb[:], in_ap=t_part_ag[:, half * H:(half + 1) * H],
                        idxs_ap=idx_cb_sb[:],
                        num_idxs=8 * 128, num_idxs_reg=8 * 128, elem_size=H,
                        elem_step=H2)
                    nc.vector.tensor_copy(out=z[:, half * H:(half + 1) * H],
                                          in_=cmb[:, 0, :])
                    for r in range(1, NC):
                        nc.vector.tensor_tensor(
                            out=z[:, half * H:(half + 1) * H],
                            in0=z[:, half * H:(half + 1) * H],
                            in1=cmb[:, r, :], op=ALU.max if half == 0 else ALU.add)
                inv_sb = const.tile([128, 1], F32)
                nc.sync.dma_start(out=inv_sb[:], in_=t_inv_cnt[:, :])
                nc.vector.tensor_scalar(out=z[:, H:H2], in0=z[:, H:H2],
                                        scalar1=inv_sb[:, 0:1], scalar2=None, op0=ALU.mult)

                # ---------------- head ----------------
                l1w = sb_w.tile([128, (H2 // 128) * H], BF16, tag="l1w")
                for k in range(H2 // 128):
                    nc.sync.dma_start(out=l1w[:, k * H:(k + 1) * H],
                                      in_=t_lin1W[k * 128:(k + 1) * 128, :])
                l1b = sb_w.tile([128, H], F32, tag="l1b")
                nc.sync.dma_start(out=l1b[:], in_=t_lin1b[:, :])
                z1 = ps_big.tile([128, H], F32, tag="pbig")
                for k in range(H2 // 128):
                    pt = ps_t.tile([128, 128], F32, tag="pt")
                    nc.tensor.matmul(pt[:], z[:, k * 128:(k + 1) * 128], ident[:],
                                     is_transpose=True, start=True, stop=True)
                    zT = sb_win.tile([128, 128], BF16, tag="zT")
                    nc.vector.tensor_copy(out=zT[:], in_=pt[:])
                    nc.tensor.matmul(z1[:], zT[:], l1w[:, k * H:(k + 1) * H],
                                     start=(k == 0), stop=(k == H2 // 128 - 1))
                z1f = sb_win.tile([128, H], F32, tag="z1f")
                nc.vector.tensor_tensor(out=z1f[:], in0=z1[:], in1=l1b[:], op=ALU.add)
                nc.scalar.activation(z1f[:], z1f[:], AF.Tanh)

                l2w = sb_w.tile([128, CH * 16], BF16, tag="l2w")
                for k in range(CH):
                    nc.sync.dma_start(out=l2w[:, k * 16:(k + 1) * 16],
                                      in_=t_lin2W[k * 128:(k + 1) * 128, :])
                l2b = sb_w.tile([128, 16], F32, tag="l2b")
                nc.sync.dma_start(out=l2b[:], in_=t_lin2b[:, :])
                z2 = ps_sm.tile([128, 16], F32, tag="psm")
                for k in range(CH):
                    pt = ps_t.tile([128, 128], F32, tag="pt")
                    nc.tensor.matmul(pt[:], z1f[:, k * 128:(k + 1) * 128], ident[:],
                                     is_transpose=True, start=True, stop=True)
                    zT = sb_win.tile([128, 128], BF16, tag="zT")
                    nc.vector.tensor_copy(out=zT[:], in_=pt[:])
                    nc.tensor.matmul(z2[:], zT[:], l2w[:, k * 16:(k + 1) * 16],
                                     start=(k == 0), stop=(k == CH - 1))
                logits = sb_win.tile([128, 16], F32, tag="lg")
                nc.vector.tensor_tensor(out=logits[:], in0=z2[:], in1=l2b[:], op=ALU.add)
                nc.vector.tensor_scalar(out=logits[:, cfg.C:16], in0=logits[:, cfg.C:16],
                                        scalar1=0.0, scalar2=-1e30, op0=ALU.mult, op1=ALU.add)
                mx = sb_win.tile([128, 1], F32, tag="mx")
                nc.vector.tensor_reduce(out=mx[:], in_=logits[:],
                                        axis=mybir.AxisListType.X, op=ALU.max)
                sh = sb_win.tile([128, 16], F32, tag="sh")
                nc.vector.tensor_scalar(out=sh[:], in0=logits[:], scalar1=mx[:, 0:1],
                                        scalar2=None, op0=ALU.subtract)
                ex = sb_win.tile([128, 16], F32, tag="ex")
                nc.scalar.activation(ex[:], sh[:], AF.Exp)
                sm = sb_win.tile([128, 1], F32, tag="sm")
                nc.vector.tensor_reduce(out=sm[:], in_=ex[:],
                                        axis=mybir.AxisListType.X, op=ALU.add)
                lsm = sb_win.tile([128, 1], F32, tag="lsm")
                nc.scalar.activation(lsm[:], sm[:], AF.Ln)
                outt = sb_win.tile([128, 16], F32, tag="outt")
                nc.vector.tensor_scalar(out=outt[:], in0=sh[:], scalar1=lsm[:, 0:1],
                                        scalar2=None, op0=ALU.subtract)
                nc.sync.dma_start(out=t_out[:, :], in_=outt[:])

    nc.compile()
    return nc


# ---------------------------------------------------------------------------
# entry point
# ---------------------------------------------------------------------------

_CACHE = {}


def run(cfg, x, edge_index, batch_index, params):
    in_maps = preprocess(cfg, x, edge_index, batch_index, params)
    key = (cfg.N, cfg.E, cfg.G, cfg.F_IN, cfg.H, cfg.C, cfg.S0, cfg.SG)
    if key not in _CACHE:
        _CACHE[key] = build_kernel(cfg)
    nc = _CACHE[key]
    res = bass_utils.run_bass_kernel_spmd(nc, in_maps, core_ids=list(range(cfg.NC)))
    out = np.concatenate(
        [res.results[c]["out"][:cfg.GPC, :cfg.C] for c in range(cfg.NC)], axis=0)
    return out.astype(np.float32), res


def kernel(**inputs):
    x = np.asarray(inputs["x"], np.float32)
    edge_index = np.asarray(inputs["edge_index"], np.int64)
    batch_index = np.asarray(inputs["batch_index"], np.int64)
    cfg = Cfg(N=x.shape[0], E=edge_index.shape[1], G=1000,
              F_IN=x.shape[1], H=512, C=10)
    out, _ = run(cfg, x, edge_index, batch_index, inputs)
    return out



# revision 2
# speedup vs baseline: 67.9577x; 67.9577x over previous
"""Distributed 4-layer GAT for 8 Trainium2 NeuronCores (Bass/Tile).

Sharding: nodes partitioned across 8 cores (dst-sharding, 12500/core); edges
co-located with their dst, grouped into 128-dst windows; per-layer AllGather
of projected features h_ext (bf16, row = [h(512) | as | pad] = 640 elems);
aggregation = per-edge-tile matmuls with one-hot(dst)*w stationaries
accumulating in PSUM; softmax without max-subtraction (logits are small,
self-loops keep denominators positive); max pooling via transpose-mode
dma_gather, sum pooling via one-hot matmul; small AllGather of per-core
pooling partials + remap-gather combine; per-core MLP head on 125 graphs.
"""
import warnings

warnings.filterwarnings("ignore")

import numpy as np
import ml_dtypes

import concourse.bacc as bacc
import concourse.tile as tile
import concourse.bass as bass
import concourse.mybir as mybir
from concourse import bass_utils
from concourse.masks import make_identity

BF16 = mybir.dt.bfloat16
F32 = mybir.dt.float32
I16 = mybir.dt.int16
AF = mybir.ActivationFunctionType
ALU = mybir.AluOpType

NEG_SLOPE = 0.2


class Cfg:
    def __init__(self, N, E, G, F_IN, H, C, NC=8):
        self.N, self.E, self.G, self.F_IN, self.H, self.C, self.NC = N, E, G, F_IN, H, C, NC
        assert N % NC == 0 and G % NC == 0
        self.NPC = N // NC
        self.NPD = ((self.NPC + 127) // 128) * 128
        self.W = self.NPD // 128
        self.KC = H // 128
        self.KC1 = F_IN // 128
        self.ROW = H + 128                       # gather row elems (bf16)
        assert (self.ROW * 2) % 256 == 0
        self.TROWS = NC * self.NPD
        self.NQ = 4
        assert self.TROWS % self.NQ == 0
        self.QROWS = self.TROWS // self.NQ
        assert self.QROWS <= 32768
        self.TPQ = 2                             # tiles per (window, quarter)
        self.SPW = self.NQ * self.TPQ * 128      # slots per window
        self.TPW = self.NQ * self.TPQ            # tiles per window
        self.GPC = G // NC
        self.GW = 3
        self.groups = []
        w0 = 0
        while w0 < self.W:
            gw = min(self.GW, self.W - w0)
            self.groups.append((w0, gw))
            w0 += gw
        # filled by preprocess:
        self.S0 = None
        self.SG = None
        # debug knobs
        self.LAYERS = 4
        self.DO_POOL = True
        self.DO_AG = True
        self.DO_AGG = True
        self.AGG_LVL = 4


# ---------------------------------------------------------------------------
# host-side preprocessing
# ---------------------------------------------------------------------------

def _idx_block(a):
    """int index list -> [128, n/16] int16 (wrapped in 16 partitions, x8 replicas)."""
    a = np.asarray(a, np.int64)
    n = len(a)
    assert n % 16 == 0
    assert a.max(initial=0) < 32768 and a.min(initial=0) >= 0, (a.min(), a.max())
    blk = np.zeros((16, n // 16), np.int16)
    blk[np.arange(n) % 16, np.arange(n) // 16] = a.astype(np.int16)
    return np.ascontiguousarray(np.tile(blk, (8, 1)))


def preprocess(cfg, x, edge_index, batch_index, params):
    N, E, G, H = cfg.N, cfg.E, cfg.G, cfg.H
    NC, NPC, NPD, W = cfg.NC, cfg.NPC, cfg.NPD, cfg.W
    # self-loops are handled analytically on-device; only real edges here
    src = edge_index[0].astype(np.int64)
    dst = edge_index[1].astype(np.int64)
    srow = (src // NPC) * NPD + (src % NPC)      # padded global table row

    order = np.argsort(dst, kind="stable")
    dst_s, srow_s = dst[order], srow[order]

    bidx = np.asarray(batch_index, np.int64)
    counts = np.bincount(bidx, minlength=G)

    per_core = [dict() for _ in range(NC)]
    core_meta = []
    S0_need, SG_need = 0, 0
    for c in range(NC):
        lo_n, hi_n = c * NPC, (c + 1) * NPC
        sel = (dst_s >= lo_n) & (dst_s < hi_n)
        d_loc = dst_s[sel] - lo_n
        s_row = srow_s[sel]
        quarter = s_row // cfg.QROWS
        q_loc = s_row - quarter * cfg.QROWS
        win = d_loc // 128
        dl128 = d_loc % 128

        idx_q = [np.zeros(W * cfg.TPQ * 128, np.int64) for _ in range(cfg.NQ)]
        mt0 = np.zeros((W * cfg.TPW, 128, 128), np.float32)
        ad_idx = np.zeros(W * cfg.SPW, np.int64)
        for w in range(W):
            m_w = win == w
            for q in range(cfg.NQ):
                m = m_w & (quarter == q)
                ql, dl = q_loc[m], dl128[m]
                cnt = len(ql)
                assert cnt <= cfg.TPQ * 128, f"core{c} w{w} q{q}: {cnt} edges"
                base = w * cfg.TPQ * 128
                idx_q[q][base:base + cnt] = ql
                slot0 = w * cfg.SPW + q * cfg.TPQ * 128
                ad_idx[slot0:slot0 + cnt] = dl + w * 128
                t0 = w * cfg.TPW + q * cfg.TPQ
                if cnt:
                    ii = np.arange(cnt)
                    mt0[t0 + ii // 128, ii % 128, dl] = 1.0
        pc = per_core[c]
        pc["idxq"] = [_idx_block(ix) for ix in idx_q]
        pc["idx_ad"] = _idx_block(ad_idx)
        pc["mt0"] = np.ascontiguousarray(
            mt0.transpose(1, 0, 2).reshape(128, W * cfg.TPW * 128)
        ).astype(ml_dtypes.bfloat16)

        b_loc = bidx[lo_n:hi_n]
        g_lo, g_hi = int(b_loc[0]), int(b_loc[-1])
        SG_need = max(SG_need, g_hi - g_lo + 1)
        S0_need = max(S0_need, int(np.bincount(b_loc).max()))
        core_meta.append((g_lo, g_hi, b_loc))

    S0 = ((S0_need + 31) // 32) * 32     # x4 slots/piece stays %128 and <=1024
    SG = ((SG_need + 15) // 16) * 16
    assert SG <= 256, SG
    assert 4 * S0 <= 1024, S0
    cfg.S0, cfg.SG = S0, SG
    cfg.PIECE_SLOTS = 4
    cfg.POOL_PIECES = SG // 4

    for c in range(NC):
        g_lo, g_hi, b_loc = core_meta[c]
        ng = g_hi - g_lo + 1
        pc = per_core[c]
        max_idx = np.full(SG * S0, cfg.NPC, np.int64)    # zero row (pads)
        sum_idx = np.full(SG * S0, cfg.NPC, np.int64)
        starts = np.searchsorted(b_loc, np.arange(g_lo, g_hi + 2))
        for s in range(ng):
            a, b = int(starts[s]), int(starts[s + 1])
            rows = np.arange(a, b)
            max_idx[s * S0:s * S0 + (b - a)] = rows
            if b > a:
                max_idx[s * S0 + (b - a):(s + 1) * S0] = a   # repeat first node
            sum_idx[s * S0:s * S0 + (b - a)] = rows
        pc["idx_pmax"] = _idx_block(max_idx)
        pc["idx_psum"] = _idx_block(sum_idx)

        Smat = np.zeros((W, 128, SG), np.float32)
        ii = np.arange(cfg.NPC)
        Smat[ii // 128, ii % 128, b_loc - g_lo] = 1.0
        pc["smat"] = np.ascontiguousarray(
            Smat.transpose(1, 0, 2).reshape(128, W * SG)).astype(ml_dtypes.bfloat16)

        own0 = c * cfg.GPC
        comb = np.zeros(8 * 128, np.int64)
        DUMMY_NEG, DUMMY_ZERO = NC * SG, NC * SG + 1
        for r in range(NC):
            glr, ghr, _ = core_meta[r]
            for j in range(cfg.GPC):
                g = own0 + j
                if glr <= g <= ghr:
                    comb[r * 128 + j] = r * SG + (g - glr)
                elif r == 0:
                    comb[r * 128 + j] = DUMMY_ZERO
                else:
                    comb[r * 128 + j] = DUMMY_NEG
        # empty graphs: max must come out 0 (reference: where(isfinite, 0))
        for j in range(cfg.GPC):
            if counts[own0 + j] == 0:
                for r in range(NC):
                    comb[r * 128 + j] = DUMMY_ZERO
        pc["idx_comb"] = _idx_block(comb)
        inv_cnt = np.zeros((128, 1), np.float32)
        inv_cnt[:cfg.GPC, 0] = 1.0 / np.maximum(counts[own0:own0 + cfg.GPC], 1)
        pc["inv_cnt"] = inv_cnt

        x1 = np.zeros((NPD, cfg.F_IN), np.float32)
        x1[:NPC] = x[c * NPC:(c + 1) * NPC]
        pc["x1T"] = np.ascontiguousarray(x1.T.astype(ml_dtypes.bfloat16))

    shared = {}
    for l in range(1, 5):
        Wl = np.asarray(params[f"W{l}"], np.float64)
        a_s = np.asarray(params[f"a{l}s"], np.float64)
        a_d = np.asarray(params[f"a{l}d"], np.float64)
        wext = np.concatenate([Wl, (Wl @ a_s)[:, None], (Wl @ a_d)[:, None]], 1)
        shared[f"wext{l}"] = np.ascontiguousarray(
            wext.astype(np.float32).astype(ml_dtypes.bfloat16))
        shared[f"brep{l}"] = np.ascontiguousarray(
            np.tile(np.asarray(params[f"b{l}"], np.float32)[None, :], (128, 1)))
    shared["lin1W"] = np.ascontiguousarray(
        np.asarray(params["lin1_W"], np.float32).astype(ml_dtypes.bfloat16))
    shared["lin1b"] = np.ascontiguousarray(
        np.tile(np.asarray(params["lin1_b"], np.float32)[None, :], (128, 1)))
    l2p = np.zeros((H, 16), np.float32)
    l2p[:, :cfg.C] = np.asarray(params["lin2_W"], np.float32)
    shared["lin2W"] = np.ascontiguousarray(l2p.astype(ml_dtypes.bfloat16))
    b2 = np.zeros((128, 16), np.float32)
    b2[:, :cfg.C] = np.tile(np.asarray(params["lin2_b"], np.float32)[None, :], (128, 1))
    shared["lin2b"] = b2

    in_maps = []
    for c in range(NC):
        pc = per_core[c]
        m = dict(x1T=pc["x1T"], idx_ad=pc["idx_ad"], idx_pmax=pc["idx_pmax"],
                 idx_psum=pc["idx_psum"], idx_comb=pc["idx_comb"],
                 inv_cnt=pc["inv_cnt"], smat=pc["smat"], mt0=pc["mt0"])
        for q in range(cfg.NQ):
            m[f"idxq{q}"] = pc["idxq"][q]
        m.update(shared)
        in_maps.append(m)
    return in_maps


# ---------------------------------------------------------------------------
# device kernel
# ---------------------------------------------------------------------------

def build_kernel(cfg):
    NC, W, H, ROW, NPD = cfg.NC, cfg.W, cfg.H, cfg.ROW, cfg.NPD
    H2 = 2 * H
    nc = bacc.Bacc("TRN2", target_bir_lowering=False, debug=False,
                   enable_asserts=False, num_devices=NC)

    t_x1T = nc.dram_tensor("x1T", [cfg.F_IN, NPD], BF16, kind="ExternalInput")
    t_wext, t_brep = [], []
    for l in range(1, 5):
        F_l = cfg.F_IN if l == 1 else H
        t_wext.append(nc.dram_tensor(f"wext{l}", [F_l, H + 2], BF16, kind="ExternalInput"))
        t_brep.append(nc.dram_tensor(f"brep{l}", [128, H], F32, kind="ExternalInput"))
    t_idxq = [nc.dram_tensor(f"idxq{q}", [128, W * cfg.TPQ * 8], I16, kind="ExternalInput")
              for q in range(cfg.NQ)]
    t_idx_ad = nc.dram_tensor("idx_ad", [128, W * cfg.SPW // 16], I16, kind="ExternalInput")
    t_mt0 = nc.dram_tensor("mt0", [128, W * cfg.TPW * 128], BF16, kind="ExternalInput")
    t_smat = nc.dram_tensor("smat", [128, W * cfg.SG], BF16, kind="ExternalInput")
    t_idx_pmax = nc.dram_tensor("idx_pmax", [128, cfg.SG * cfg.S0 // 16], I16, kind="ExternalInput")
    t_idx_psum = nc.dram_tensor("idx_psum", [128, cfg.SG * cfg.S0 // 16], I16, kind="ExternalInput")
    t_idx_comb = nc.dram_tensor("idx_comb", [128, 8 * 128 // 16], I16, kind="ExternalInput")
    t_inv_cnt = nc.dram_tensor("inv_cnt", [128, 1], F32, kind="ExternalInput")
    t_lin1W = nc.dram_tensor("lin1W", [H2, H], BF16, kind="ExternalInput")
    t_lin1b = nc.dram_tensor("lin1b", [128, H], F32, kind="ExternalInput")
    t_lin2W = nc.dram_tensor("lin2W", [H, 16], BF16, kind="ExternalInput")
    t_lin2b = nc.dram_tensor("lin2b", [128, 16], F32, kind="ExternalInput")
    t_out = nc.dram_tensor("out", [128, 16], F32, kind="ExternalOutput")

    t_hown = nc.dram_tensor("hown", [NPD, ROW], BF16, kind="Internal")
    t_hfull = nc.dram_tensor("hfull", [NC * NPD, ROW], BF16, kind="Internal",
                             addr_space="Shared")
    t_xnext = nc.dram_tensor("xnext", [NPD + 128, H], BF16, kind="Internal")
    t_adrep = nc.dram_tensor("adrep", [NPD, 128], BF16, kind="Internal")
    t_pool_x = nc.dram_tensor("pool_x", [NPD + 128, H], BF16, kind="Internal")
    t_part = nc.dram_tensor("part", [cfg.SG, H2], F32, kind="Internal")
    t_part_ag = nc.dram_tensor("part_ag", [NC * cfg.SG + 128, H2], F32,
                               kind="Internal", addr_space="Shared")

    NBLK = 7 if (W % 7 == 0) else 1
    TPB = W // NBLK
    NSG = (cfg.SG + 127) // 128
    CH = H // 128
    LAST_ROWS = cfg.NPC - 128 * (W - 1)   # valid rows in last window

    with tile.TileContext(nc) as tc:
        import contextlib
        with contextlib.ExitStack() as ctx:
            const = ctx.enter_context(tc.tile_pool(name="const", bufs=1))
            sb_w = ctx.enter_context(tc.tile_pool(name="weights", bufs=1))
            sb_xt = ctx.enter_context(tc.tile_pool(name="xt", bufs=2))
            sb_dense = ctx.enter_context(tc.tile_pool(name="dense", bufs=3))
            sb_g = ctx.enter_context(tc.tile_pool(name="gather", bufs=2))
            sb_mt = ctx.enter_context(tc.tile_pool(name="mt", bufs=2))
            sb_win = ctx.enter_context(tc.tile_pool(name="win", bufs=2))
            ps_big = ctx.enter_context(tc.tile_pool(name="psb", bufs=3, space="PSUM"))
            ps_sm = ctx.enter_context(tc.tile_pool(name="pssm", bufs=3, space="PSUM"))
            ps_t = ctx.enter_context(tc.tile_pool(name="pst", bufs=2, space="PSUM"))

            ones_col = const.tile([128, 1], BF16)
            nc.gpsimd.memset(ones_col[:], 1.0)
            zeros_sb = const.tile([128, H], BF16)
            nc.gpsimd.memset(zeros_sb[:], 0.0)
            ident = const.tile([128, 128], F32)
            make_identity(nc, ident[:])
            ident_bf = const.tile([128, 128], BF16)
            nc.vector.tensor_copy(out=ident_bf[:], in_=ident[:])
            as_sb = const.tile([128, W], F32)
            ad_sb = const.tile([128, W], F32)


            for l in range(1, cfg.LAYERS + 1):
                F_l = cfg.F_IN if l == 1 else H
                KCl = F_l // 128
                wsb = sb_w.tile([128, KCl * (H + 2)], BF16, tag="wsb")
                for k in range(KCl):
                    nc.sync.dma_start(out=wsb[:, k * (H + 2):(k + 1) * (H + 2)],
                                      in_=t_wext[l - 1][k * 128:(k + 1) * 128, :])
                brep = sb_w.tile([128, H], F32, tag="brep")
                nc.sync.dma_start(out=brep[:], in_=t_brep[l - 1][:, :])

                # ---------------- dense: h_ext = x @ [W | was | wad]
                for b in range(NBLK):
                    xt = sb_xt.tile([128, KCl * TPB * 128], BF16, tag="xt")
                    if l > 1:
                        for k in range(KCl):
                            nc.sync.dma_start(
                                out=xt[:, k * TPB * 128:(k + 1) * TPB * 128],
                                in_=t_xnext[b * TPB * 128:(b + 1) * TPB * 128,
                                            k * 128:(k + 1) * 128],
                                transpose=True)
                    else:
                        nc.sync.dma_start(
                            out=xt[:, 0:TPB * 128],
                            in_=t_x1T[:, b * TPB * 128:(b + 1) * TPB * 128])
                    for i in range(TPB):
                        ti = b * TPB + i

                        def lhsT_of(k):
                            return xt[:, (k * TPB + i) * 128:(k * TPB + i + 1) * 128]

                        ph = ps_big.tile([128, H], F32, tag="pbig")
                        for k in range(KCl):
                            nc.tensor.matmul(ph[:], lhsT_of(k),
                                             wsb[:, k * (H + 2):k * (H + 2) + H],
                                             start=(k == 0), stop=(k == KCl - 1))
                        psc = ps_sm.tile([128, 16], F32, tag="psm")
                        for k in range(KCl):
                            nc.tensor.matmul(psc[:, 0:2], lhsT_of(k),
                                             wsb[:, k * (H + 2) + H:(k + 1) * (H + 2)],
                                             start=(k == 0), stop=(k == KCl - 1))
                        hext = sb_dense.tile([128, H + 1], BF16, tag="hext")
                        nc.vector.tensor_copy(out=hext[:, 0:H], in_=ph[:])
                        nc.vector.tensor_copy(out=hext[:, H:H + 1], in_=psc[:, 0:1])
                        nc.vector.tensor_copy(out=as_sb[:, ti:ti + 1], in_=psc[:, 0:1])
                        nc.vector.tensor_copy(out=ad_sb[:, ti:ti + 1], in_=psc[:, 1:2])
                        nc.sync.dma_start(out=t_hown[ti * 128:(ti + 1) * 128, 0:H + 1],
                                          in_=hext[:])
                        adr = sb_dense.tile([128, 128], BF16, tag="adr")
                        nc.vector.tensor_copy(out=adr[:],
                                              in_=psc[:, 1:2].to_broadcast([128, 128]))
                        nc.sync.dma_start(out=t_adrep[ti * 128:(ti + 1) * 128, :],
                                          in_=adr[:])

                # ---------------- AllGather
                if cfg.DO_AG:
                    nc.gpsimd.collective_compute(
                        "AllGather", ALU.bypass,
                        replica_groups=[list(range(NC))],
                        ins=[t_hown[:, :]], outs=[t_hfull[:, :]])
                else:
                    for _c in range(NC):
                        nc.sync.dma_start(
                            out=t_hfull[_c * NPD:(_c + 1) * NPD, :],
                            in_=t_hown[:, :])

                # ---------------- aggregation
                dst_t = t_pool_x if l == cfg.LAYERS else t_xnext
                agg_groups = cfg.groups if cfg.DO_AGG else []
                for (w0, gw) in agg_groups:
                    gbufs = []
                    for q in range(cfg.NQ):
                        iq = sb_mt.tile([128, cfg.GW * cfg.TPQ * 8], I16, tag="iq")
                        nc.sync.dma_start(
                            out=iq[:, 0:gw * cfg.TPQ * 8],
                            in_=t_idxq[q][:, w0 * cfg.TPQ * 8:(w0 + gw) * cfg.TPQ * 8])
                        gq = sb_g.tile([128, cfg.GW * cfg.TPQ, ROW], BF16, tag=f"g{q}")
                        nc.gpsimd.dma_gather(
                            out_ap=gq[:, 0:gw * cfg.TPQ, :],
                            in_ap=t_hfull[q * cfg.QROWS:(q + 1) * cfg.QROWS, :],
                            idxs_ap=iq[:, 0:gw * cfg.TPQ * 8],
                            num_idxs=gw * cfg.TPQ * 128,
                            num_idxs_reg=gw * cfg.TPQ * 128,
                            elem_size=ROW)
                        gbufs.append(gq)
                    adg = sb_g.tile([128, cfg.GW * cfg.TPW, 128], BF16, tag="adg")
                    ia = sb_mt.tile([128, cfg.GW * cfg.SPW // 16], I16, tag="ia")
                    nc.sync.dma_start(
                        out=ia[:, 0:gw * cfg.SPW // 16],
                        in_=t_idx_ad[:, w0 * cfg.SPW // 16:(w0 + gw) * cfg.SPW // 16])
                    for lw in range(gw):
                        # dma_gather is limited to <=1024 indices per call
                        nc.gpsimd.dma_gather(
                            out_ap=adg[:, lw * cfg.TPW:(lw + 1) * cfg.TPW, :],
                            in_ap=t_adrep[:, :],
                            idxs_ap=ia[:, lw * cfg.SPW // 16:(lw + 1) * cfg.SPW // 16],
                            num_idxs=cfg.SPW, num_idxs_reg=cfg.SPW,
                            elem_size=128)
                    mt0g = sb_mt.tile([128, cfg.GW * cfg.TPW * 128], BF16, tag="mt0g")
                    nc.sync.dma_start(
                        out=mt0g[:, 0:gw * cfg.TPW * 128],
                        in_=t_mt0[:, w0 * cfg.TPW * 128:(w0 + gw) * cfg.TPW * 128])

                    TPW1 = cfg.TPW + 1
                    if cfg.AGG_LVL < 2:
                        continue
                    wbuf = sb_win.tile([128, cfg.GW * TPW1, 1], F32, tag="wbuf")
                    for lw in range(gw):
                        for q in range(cfg.NQ):
                            o0 = lw * TPW1 + q * cfg.TPQ
                            nc.vector.tensor_tensor(
                                out=wbuf[:, o0:o0 + cfg.TPQ, :],
                                in0=gbufs[q][:, lw * cfg.TPQ:(lw + 1) * cfg.TPQ, H:H + 1],
                                in1=adg[:, lw * cfg.TPW + q * cfg.TPQ:
                                        lw * cfg.TPW + (q + 1) * cfg.TPQ, 0:1],
                                op=ALU.add)
                        nc.vector.tensor_tensor(
                            out=wbuf[:, lw * TPW1 + cfg.TPW, 0:1],
                            in0=as_sb[:, w0 + lw:w0 + lw + 1],
                            in1=ad_sb[:, w0 + lw:w0 + lw + 1], op=ALU.add)
                    wb2 = sb_win.tile([128, cfg.GW * TPW1, 1], F32, tag="wb2")
                    nc.vector.tensor_scalar(
                        out=wb2[:, 0:gw * TPW1, :], in0=wbuf[:, 0:gw * TPW1, :],
                        scalar1=NEG_SLOPE, scalar2=None, op0=ALU.mult)
                    nc.vector.tensor_tensor(
                        out=wbuf[:, 0:gw * TPW1, :], in0=wbuf[:, 0:gw * TPW1, :],
                        in1=wb2[:, 0:gw * TPW1, :], op=ALU.max)
                    nc.scalar.activation(wbuf[:, 0:gw * TPW1, :],
                                         wbuf[:, 0:gw * TPW1, :], AF.Exp)

                    if cfg.AGG_LVL < 3:
                        continue
                    for lw in range(gw):
                        w = w0 + lw
                        po = ps_big.tile([128, H], F32, tag="pbig")
                        pd = ps_sm.tile([128, 16], F32, tag="psm")
                        # self-loop first: diag(w_self) @ h_own[window]
                        hw = sb_dense.tile([128, H], BF16, tag="hw")
                        nc.sync.dma_start(out=hw[:],
                                          in_=t_hown[w * 128:(w + 1) * 128, 0:H])
                        mts = sb_mt.tile([128, 128], BF16, tag="mt")
                        nc.vector.tensor_scalar(
                            out=mts[:], in0=ident_bf[:],
                            scalar1=wbuf[:, lw * TPW1 + cfg.TPW, 0:1],
                            scalar2=None, op0=ALU.mult)
                        nc.tensor.matmul(po[:], mts[:], hw[:], start=True, stop=False)
                        nc.tensor.matmul(pd[:, 0:1], mts[:], ones_col[:],
                                         start=True, stop=False)
                        for t in range(cfg.TPW):
                            q, sub = t // cfg.TPQ, t % cfg.TPQ
                            mt = sb_mt.tile([128, 128], BF16, tag="mt")
                            nc.vector.tensor_scalar(
                                out=mt[:],
                                in0=mt0g[:, (lw * cfg.TPW + t) * 128:
                                         (lw * cfg.TPW + t + 1) * 128],
                                scalar1=wbuf[:, lw * TPW1 + t, 0:1],
                                scalar2=None, op0=ALU.mult)
                            nc.tensor.matmul(po[:], mt[:],
                                             gbufs[q][:, lw * cfg.TPQ + sub, 0:H],
                                             start=False, stop=(t == cfg.TPW - 1))
                            nc.tensor.matmul(pd[:, 0:1], mt[:], ones_col[:],
                                             start=False, stop=(t == cfg.TPW - 1))
                        if cfg.AGG_LVL < 4:
                            continue
                        rec = sb_win.tile([128, 1], F32, tag="rec")
                        nc.vector.reciprocal(rec[:], pd[:, 0:1])
                        xn = sb_win.tile([128, H], BF16, tag="xn")
                        nc.vector.scalar_tensor_tensor(
                            out=xn[:], in0=po[:], scalar=rec[:, 0:1], in1=brep[:],
                            op0=ALU.mult, op1=ALU.add)
                        nc.scalar.activation(xn[:], xn[:], AF.Tanh)
                        nr = LAST_ROWS if w == W - 1 else 128
                        nc.sync.dma_start(out=dst_t[w * 128:w * 128 + nr, 0:H],
                                          in_=xn[0:nr, :])
                if not cfg.DO_AGG:
                    for w in range(W):
                        nc.sync.dma_start(out=dst_t[w * 128:(w + 1) * 128, 0:H],
                                          in_=t_hown[w * 128:(w + 1) * 128, 0:H])
                nc.sync.dma_start(out=dst_t[cfg.NPC:cfg.NPC + 128, 0:H],
                                  in_=zeros_sb[:])

            if not cfg.DO_POOL:
                dbg = sb_win.tile([128, 16], F32, tag="outt")
                nc.gpsimd.memset(dbg[:], 0.0)
                nc.sync.dma_start(out=t_out[:, :], in_=dbg[:])
            else:
                # ---------------- pooling ----------------
                psums = []
                for _sgc in range(NSG):
                    pss_t = ps_big.tile([128, H], F32, tag="pbig", name=f"pss{_sgc}")
                    psums.append(pss_t)
                for i in range(W):
                    smt = sb_dense.tile([128, cfg.SG], BF16, tag="smt")
                    nc.sync.dma_start(out=smt[:], in_=t_smat[:, i * cfg.SG:(i + 1) * cfg.SG])
                    xtile = sb_dense.tile([128, H], BF16, tag="hext")
                    nc.sync.dma_start(out=xtile[:],
                                      in_=t_pool_x[i * 128:(i + 1) * 128, 0:H])
                    for sgc in range(NSG):
                        cols = min(128, cfg.SG - sgc * 128)
                        nc.tensor.matmul(
                            psums[sgc][0:cols, :],
                            smt[:, sgc * 128:sgc * 128 + cols],
                            xtile[:], start=(i == 0), stop=(i == W - 1))
                for sgc in range(NSG):
                    cols = min(128, cfg.SG - sgc * 128)
                    ssb = sb_win.tile([128, H], F32, tag="ssb")
                    nc.vector.tensor_copy(out=ssb[0:cols, :], in_=psums[sgc][0:cols, :])
                    nc.sync.dma_start(out=t_part[sgc * 128:sgc * 128 + cols, H:H2],
                                      in_=ssb[0:cols, :])

                idx_pm_sb = const.tile([128, cfg.SG * cfg.S0 // 16], I16)
                nc.sync.dma_start(out=idx_pm_sb[:], in_=t_idx_pmax[:, :])
                maxT = const.tile([128, CH * cfg.SG], F32)
                PS = cfg.PIECE_SLOTS
                for piece in range(cfg.POOL_PIECES):
                    n_idx = PS * cfg.S0
                    gt = sb_g.tile([128, CH, n_idx], BF16, tag="g0")
                    nc.gpsimd.dma_gather(
                        out_ap=gt[:],
                        in_ap=t_pool_x[:, :],
                        idxs_ap=idx_pm_sb[:, piece * n_idx // 16:(piece + 1) * n_idx // 16],
                        num_idxs=n_idx, num_idxs_reg=n_idx,
                        elem_size=H, transpose=True)
                    for ch in range(CH):
                        nc.vector.tensor_reduce(
                            out=maxT[:, ch * cfg.SG + piece * PS:
                                     ch * cfg.SG + (piece + 1) * PS],
                            in_=gt[:, ch, :].rearrange("p (s j) -> p s j", j=cfg.S0),
                            axis=mybir.AxisListType.X, op=ALU.max)
                for sgc in range(NSG):
                    cols = min(128, cfg.SG - sgc * 128)
                    for ch in range(CH):
                        pt = ps_t.tile([128, 128], F32, tag="pt")
                        nc.tensor.matmul(
                            pt[0:cols, 0:128],
                            maxT[:, ch * cfg.SG + sgc * 128:ch * cfg.SG + sgc * 128 + cols],
                            ident[:], is_transpose=True, start=True, stop=True)
                        mtile = sb_win.tile([128, 128], F32, tag="mtile")
                        nc.vector.tensor_copy(out=mtile[0:cols, :], in_=pt[0:cols, 0:128])
                        nc.sync.dma_start(
                            out=t_part[sgc * 128:sgc * 128 + cols, ch * 128:(ch + 1) * 128],
                            in_=mtile[0:cols, :])

                if cfg.DO_AG:
                    nc.gpsimd.collective_compute(
                        "AllGather", ALU.bypass,
                        replica_groups=[list(range(NC))],
                        ins=[t_part[:, :]], outs=[t_part_ag[0:NC * cfg.SG, :]])
                else:
                    for _c in range(NC):
                        nc.sync.dma_start(
                            out=t_part_ag[_c * cfg.SG:(_c + 1) * cfg.SG, :],
                            in_=t_part[:, :])
                zr = sb_win.tile([128, H2], F32, tag="zr")
                nc.gpsimd.memset(zr[:, 0:H], -1e30)
                nc.gpsimd.memset(zr[:, H:H2], 0.0)
                nc.sync.dma_start(out=t_part_ag[NC * cfg.SG:NC * cfg.SG + 1, :],
                                  in_=zr[0:1, :])
                zrz = sb_win.tile([128, H2], F32, tag="zrz")
                nc.gpsimd.memset(zrz[:], 0.0)
                nc.sync.dma_start(out=t_part_ag[NC * cfg.SG + 1:NC * cfg.SG + 2, :],
                                  in_=zrz[0:1, :])

                idx_cb_sb = const.tile([128, 8 * 128 // 16], I16)
                nc.sync.dma_start(out=idx_cb_sb[:], in_=t_idx_comb[:, :])
                z = sb_win.tile([128, H2], F32, tag="z")
                for half in range(2):
                    cmb = sb_g.tile([128, 8, H], F32, tag="g1")
                    nc.gpsimd.dma_gather(
                        out_ap=cmb[:], in_ap=t_part_ag[:, half * H:(half + 1) * H],
                        idxs_ap=idx_cb_sb[:],
                        num_idxs=8 * 128, num_idxs_reg=8 * 128, elem_size=H,
                        elem_step=H2)
                    nc.vector.tensor_copy(out=z[:, half * H:(half + 1) * H],
                                          in_=cmb[:, 0, :])
                    for r in range(1, NC):
                        nc.vector.tensor_tensor(
                            out=z[:, half * H:(half + 1) * H],
                            in0=z[:, half * H:(half + 1) * H],
                            in1=cmb[:, r, :], op=ALU.max if half == 0 else ALU.add)
                inv_sb = const.tile([128, 1], F32)
                nc.sync.dma_start(out=inv_sb[:], in_=t_inv_cnt[:, :])
                nc.vector.tensor_scalar(out=z[:, H:H2], in0=z[:, H:H2],
                                        scalar1=inv_sb[:, 0:1], scalar2=None, op0=ALU.mult)

                # ---------------- head ----------------
                l1w = sb_w.tile([128, (H2 // 128) * H], BF16, tag="l1w")
                for k in range(H2 // 128):
                    nc.sync.dma_start(out=l1w[:, k * H:(k + 1) * H],
                                      in_=t_lin1W[k * 128:(k + 1) * 128, :])
                l1b = sb_w.tile([128, H], F32, tag="l1b")
                nc.sync.dma_start(out=l1b[:], in_=t_lin1b[:, :])
                z1 = ps_big.tile([128, H], F32, tag="pbig")
                for k in range(H2 // 128):
                    pt = ps_t.tile([128, 128], F32, tag="pt")
                    nc.tensor.matmul(pt[:], z[:, k * 128:(k + 1) * 128], ident[:],
                                     is_transpose=True, start=True, stop=True)
                    zT = sb_win.tile([128, 128], BF16, tag="zT")
                    nc.vector.tensor_copy(out=zT[:], in_=pt[:])
                    nc.tensor.matmul(z1[:], zT[:], l1w[:, k * H:(k + 1) * H],
                                     start=(k == 0), stop=(k == H2 // 128 - 1))
                z1f = sb_win.tile([128, H], F32, tag="z1f")
                nc.vector.tensor_tensor(out=z1f[:], in0=z1[:], in1=l1b[:], op=ALU.add)
                nc.scalar.activation(z1f[:], z1f[:], AF.Tanh)

                l2w = sb_w.tile([128, CH * 16], BF16, tag="l2w")
                for k in range(CH):
                    nc.sync.dma_start(out=l2w[:, k * 16:(k + 1) * 16],
                                      in_=t_lin2W[k * 128:(k + 1) * 128, :])
                l2b = sb_w.tile([128, 16], F32, tag="l2b")
                nc.sync.dma_start(out=l2b[:], in_=t_lin2b[:, :])
                z2 = ps_sm.tile([128, 16], F32, tag="psm")
                for k in range(CH):
                    pt = ps_t.tile([128, 128], F32, tag="pt")
                    nc.tensor.matmul(pt[:], z1f[:, k * 128:(k + 1) * 128], ident[:],
                                     is_transpose=True, start=True, stop=True)
                    zT = sb_win.tile([128, 128], BF16, tag="zT")
                    nc.vector.tensor_copy(out=zT[:], in_=pt[:])
                    nc.tensor.matmul(z2[:], zT[:], l2w[:, k * 16:(k + 1) * 16],
                                     start=(k == 0), stop=(k == CH - 1))
                logits = sb_win.tile([128, 16], F32, tag="lg")
                nc.vector.tensor_tensor(out=logits[:], in0=z2[:], in1=l2b[:], op=ALU.add)
                nc.vector.tensor_scalar(out=logits[:, cfg.C:16], in0=logits[:, cfg.C:16],
                                        scalar1=0.0, scalar2=-1e30, op0=ALU.mult, op1=ALU.add)
                mx = sb_win.tile([128, 1], F32, tag="mx")
                nc.vector.tensor_reduce(out=mx[:], in_=logits[:],
                                        axis=mybir.AxisListType.X, op=ALU.max)
                sh = sb_win.tile([128, 16], F32, tag="sh")
                nc.vector.tensor_scalar(out=sh[:], in0=logits[:], scalar1=mx[:, 0:1],
                                        scalar2=None, op0=ALU.subtract)
                ex = sb_win.tile([128, 16], F32, tag="ex")
                nc.scalar.activation(ex[:], sh[:], AF.Exp)
                sm = sb_win.tile([128, 1], F32, tag="sm")
                nc.vector.tensor_reduce(out=sm[:], in_=ex[:],
                                        axis=mybir.AxisListType.X, op=ALU.add)
                lsm = sb_win.tile([128, 1], F32, tag="lsm")
                nc.scalar.activation(lsm[:], sm[:], AF.Ln)
                outt = sb_win.tile([128, 16], F32, tag="outt")
                nc.vector.tensor_scalar(out=outt[:], in0=sh[:], scalar1=lsm[:, 0:1],
                                        scalar2=None, op0=ALU.subtract)
                nc.sync.dma_start(out=t_out[:, :], in_=outt[:])

    nc.compile()
    return nc


# ---------------------------------------------------------------------------
# entry point
# ---------------------------------------------------------------------------

_CACHE = {}


def run(cfg, x, edge_index, batch_index, params):
    in_maps = preprocess(cfg, x, edge_index, batch_index, params)
    key = (cfg.N, cfg.E, cfg.G, cfg.F_IN, cfg.H, cfg.C, cfg.S0, cfg.SG)
    if key not in _CACHE:
        _CACHE[key] = build_kernel(cfg)
    nc = _CACHE[key]
    res = bass_utils.run_bass_kernel_spmd(nc, in_maps, core_ids=list(range(cfg.NC)))
    out = np.concatenate(
        [res.results[c]["out"][:cfg.GPC, :cfg.C] for c in range(cfg.NC)], axis=0)
    return out.astype(np.float32), res


def kernel(**inputs):
    x = np.asarray(inputs["x"], np.float32)
    edge_index = np.asarray(inputs["edge_index"], np.int64)
    batch_index = np.asarray(inputs["batch_index"], np.int64)
    cfg = Cfg(N=x.shape[0], E=edge_index.shape[1], G=1000,
              F_IN=x.shape[1], H=512, C=10)
    out, _ = run(cfg, x, edge_index, batch_index, inputs)
    return out

